# revision 1
# baseline (speedup 1.0000x reference)
"""Causal GQA attention (paged-KV prefill) distributed over 8 TRN2 NeuronCores.

Problem: q [4096,16,128], k/v [4096,4,128] packed as B=2 sequences of S=2048,
KV-cache scatter via slot_mapping then causal attention (GQA group 4).

Sharding: 8 cores = (B=2) x (Hkv=4). Core c handles batch c//4, kv-head c%4
with its 4 query heads. No cross-core communication needed.

Per-core kernel (Bass/Tile):
  - host pre-casts shards to bf16 and pre-tiles them to the SBUF-native
    [s%128, s//128, d] layout (contiguous 4KB DMA rows, full line rate);
    v arrives padded to 129 cols with its ones column baked in
  - K and Q reach SBUF through xbar DMA-transposes reading DRAM directly
    (no staging loads), split so the first chunk's operands arrive first
  - scores^T tile [k=128, q<=512] = kT_tile.T @ qT_chunk on TensorE (bf16),
    causally trimmed: diagonal-band tiles only compute the valid query range
  - exp(scale*s) on ScalarE straight out of PSUM, one call per 2-tile
    group; every 3rd fully-causal group instead computes exp on VectorE as
    an int16 affine whose bits are bf16(exp(x)) (Schraudolph), offloading
    the otherwise-saturated ScalarE (~4e-3 extra end-to-end error)
  - causal diag blocks masked via 0/1 triangular mult on VectorE
  - out accumulation: psum_o[q=128, 129] += probT_tile.T @ [v_tile | 1],
    the 129th column accumulates the softmax denominator for free; two
    q-subblocks pack into one PSUM bank ([128, 258]). Each bank's first
    AV opens the 2KB zero region with start=True; the bank's second
    accumulator then overwrites its has_written=0 region (two interleaved
    start-groups in one bank would clear each other's has_written bits)
  - normalize: copy PSUM->SBUF early (frees the bank), then VectorE
    reciprocal + tensor_scalar, DMA out f32
  - emission is software-pipelined with a 4-unit scores lookahead (PE
    always has queued score matmuls while ScalarE/VectorE exp a group),
    q-chunks run largest-first, the tri mask loads after the critical
    transposes (no xbar-mode switch ahead of them), and the lookahead
    drains before the final chunk to shorten the end-of-kernel tail

PSUM budget (8 banks): scores [128,1024] x3 bufs = 6, packed out
accumulators [128,258] x2 tags x1 buf = 2.
"""

import os
import sys

import numpy as np

for _p in ("/opt/trn_rl_repo",):
    if os.path.isdir(_p) and _p not in sys.path:
        sys.path.insert(0, _p)

import ml_dtypes  # noqa: E402

from concourse import bass, bacc, mybir, tile  # noqa: E402
from concourse.bass_utils import run_bass_kernel_spmd  # noqa: E402

B, S, H, HKV, D = 2, 2048, 16, 4, 128
GRP = H // HKV  # query heads per kv head
NCORES = 8
ST = S // 128  # 16 k-tiles of 128
QB = S // 512  # 4 q-chunks of 512
SCALE = 0.08838834764831845  # 1/sqrt(128)
# Schraudolph-in-bf16-bits exp on DVE: int16 bits = A16*(scale*s) + B16
# approximate bf16(exp(scale*s)) to ~3% per element. Applied to every
# DVE_EVERY-th fully-below-diagonal score group to offload the saturated
# ScalarE; softmax renormalization cancels most of the per-element error
# (measured end-to-end ~4e-3 on top of the ~3e-3 bf16 baseline).
import math as _math

A16S = (2.0**7) / _math.log(2.0) * SCALE
B16 = 127.0 * 2**7 - 366393.0 / 2**16
DVE_EVERY = 3

F32 = mybir.dt.float32
BF16 = mybir.dt.bfloat16
I16 = mybir.dt.int16

_CACHED_NC = None


def _build_graph():
    nc = bacc.Bacc(
        "TRN2", target_bir_lowering=False, debug=False, num_devices=NCORES
    )
    # host pre-tiles shards to the SBUF-native layout [s%128, s//128, d]
    # (4KB contiguous DMA rows); v arrives with its ones column baked in
    q_ext = nc.declare_dram_parameter("q", [GRP, 128, ST, D], BF16, isOutput=False)
    k_ext = nc.declare_dram_parameter("k", [128, ST, D], BF16, isOutput=False)
    v_ext = nc.declare_dram_parameter("v", [128, ST, D + 1], BF16, isOutput=False)
    tri_ext = nc.declare_dram_parameter("tri", [128, 128], BF16, isOutput=False)
    out_ext = nc.declare_dram_parameter("out", [S, GRP, D], F32, isOutput=True)

    with tile.TileContext(nc) as tc:
        with (
            tc.tile_pool(name="const", bufs=1) as constp,
            tc.tile_pool(name="kv", bufs=1) as kvp,
            tc.tile_pool(name="prob", bufs=10) as probp,
            tc.tile_pool(name="osb", bufs=6) as osbp,
            tc.tile_pool(name="small", bufs=16) as smallp,
            tc.tile_pool(name="ps_s", bufs=3, space=bass.MemorySpace.PSUM) as pss,
            tc.tile_pool(name="ps_o", bufs=1, space=bass.MemorySpace.PSUM) as pso,
        ):
            # 0/1 lower-allowed mask for diagonal blocks: tri[kk, qq] = kk <= qq
            tri = constp.tile([128, 128], BF16)

            # warm the exp table set while input DMAs run
            warm = smallp.tile([128, 1], F32, tag="warm")
            nc.vector.memset(warm[:], 0.0)
            nc.scalar.activation(
                warm[:], warm[:], mybir.ActivationFunctionType.Exp
            )
            # warm the PE clock (HAM ramps over ~3.4us of sustained
            # activity): stream dummy matmuls while the first transposes
            # are still in flight so the real scores start at full rate
            wmm = smallp.tile([128, 8], BF16, tag="wmm")
            nc.vector.memset(wmm[:], 0.0)
            wps = pss.tile([128, 1024], F32, tag="s", name="wps")
            for _ in range(75):
                nc.tensor.matmul(
                    wps[:8, 0:8],
                    wmm[:],
                    wmm[:],
                    start=True,
                    stop=True,
                )

            # Inputs arrive bf16 pre-tiled (host does layout prep during
            # sharding), so loads are plain full-line-rate HWDGE copies and
            # only the xbar transposes (head_dim onto partitions) remain.
            # Copy->transpose xbar-mode switches serialize the DMA pool, so
            # copies and transposes are batched, not interleaved.
            kr = k_ext.ap()
            vr = v_ext.ap()
            qr = q_ext.ap()

            v_aug = kvp.tile([128, ST, 129], BF16, tag="vaug")
            kT = kvp.tile([128, ST, 128], BF16, tag="kT")
            kTf = kT[:].rearrange("d st s0 -> d (st s0)")  # [128, 2048]
            qTs = [None] * GRP
            qTfs = [None] * GRP
            for h in range(GRP):
                qTs[h] = kvp.tile(
                    [128, ST, 128], BF16, tag=f"qT{h}", name="qT"
                )
                qTfs[h] = qTs[h][:].rearrange("d st s0 -> d (st s0)")

            HF = ST // 2

            def half(ap3, lo, hi):
                return ap3[:, lo:hi, :]

            # Inputs are bf16 pre-tiled, so the xbar transposes read DRAM
            # directly -- no staging loads, one DMA hop. The first chunk
            # (h0, qb3) needs qT0 cols 1536+ (q0 k-tiles 8-15) and kT tiles
            # 0-7 first, so those halves transpose first.
            QT3 = 3 * ST // 4
            nc.sync.dma_start_transpose(
                out=half(qTs[0][:], QT3, ST), in_=half(qr[0], QT3, ST)
            )
            nc.sync.dma_start_transpose(
                out=half(kT[:], 0, HF), in_=half(kr, 0, HF)
            )
            nc.sync.dma_start_transpose(
                out=half(kT[:], HF, ST), in_=half(kr, HF, ST)
            )
            nc.sync.dma_start(v_aug[:], vr)
            nc.sync.dma_start(tri[:], tri_ext.ap())
            nc.sync.dma_start_transpose(
                out=half(qTs[0][:], HF, QT3), in_=half(qr[0], HF, QT3)
            )
            t_w1 = nc.sync.dma_start_transpose(
                out=half(qTs[0][:], 0, HF), in_=half(qr[0], 0, HF)
            )
            v_augf = v_aug[:].rearrange("s0 st d -> s0 (st d)")

            wave2_state = {"t_prev": t_w1}

            def emit_wave2(h):
                t = nc.sync.dma_start_transpose(out=qTs[h][:], in_=qr[h])
                tile.add_dep_helper(
                    t.ins,
                    wave2_state["t_prev"].ins,
                    reason="wave-2 transpose ordering",
                )
                wave2_state["t_prev"] = t

            outr = out_ext.ap().rearrange(
                "(qb j s0) h d -> qb h s0 j d", j=4, s0=128
            )

            def po_slice(po, j):
                t = po[0] if j < 2 else po[1]
                off = 129 * (j % 2)
                return t[:, off : off + 129]

            def emit_scores(h, qb, g):
                """Issue the two trimmed score matmuls for k-tile pair g."""
                kbs = (2 * g, 2 * g + 1)
                trims = [max(0, kb - 4 * qb) * 128 for kb in kbs]
                widths = [512 - t for t in trims]
                same_bank = widths[0] + widths[1] <= 512
                # same-bank trimmed pair packs contiguously: tile0's
                # start=True pending-zeroes the whole bank, tile1 writes
                # its slice with start=False (overwrite of pending bytes),
                # so the exp reads one contiguous hole-free range
                offs = [0, widths[0]] if same_bank else [0, 512]
                ps = pss.tile([128, 1024], F32, tag="s", name="ps")
                for i in (0, 1):
                    kb, t, w, o = kbs[i], trims[i], widths[i], offs[i]
                    nc.tensor.matmul(
                        ps[:, o : o + w],
                        kTf[:, kb * 128 : (kb + 1) * 128],
                        qTfs[h][:, qb * 512 + t : (qb + 1) * 512],
                        start=(not same_bank) or i == 0,
                        stop=(not same_bank) or i == 1,
                    )
                return (ps, kbs, trims, offs, widths)

            def emit_rest(h, qb, g, po, scored):
                """exp + mask + AV accumulation for a scored group; on the
                chunk's last group also normalize + store."""
                ps, kbs, trims, offs, widths = scored
                # DVE-exp: every 3rd fully-causal group (phase-tuned)
                full = kbs[1] < 4 * qb  # both tiles fully below the diagonal
                eligible = full
                if eligible:
                    exp_state["ctr"] += 1
                if eligible and exp_state["ctr"] % 3 == 1:
                    # offload this group's exp to DVE (Schraudolph bf16 bits)
                    i16 = probp.tile([128, 1024], I16, tag="p", name="probTi")
                    nc.vector.tensor_scalar(
                        i16[:],
                        ps[:],
                        A16S,
                        B16,
                        mybir.AluOpType.mult,
                        mybir.AluOpType.add,
                    )
                    probT = i16.bitcast(BF16)
                else:
                    probT_t = probp.tile(
                        [128, 1024], BF16, tag="p", name="probT"
                    )
                    probT = probT_t[:]
                    total_w = offs[1] + widths[1]  # contiguous, hole-free
                    nc.scalar.activation(
                        probT[:, 0:total_w],
                        ps[:, 0:total_w],
                        mybir.ActivationFunctionType.Exp,
                        scale=SCALE,
                    )
                started_banks = set()
                for i in (0, 1):
                    kb, t, o = kbs[i], trims[i], offs[i]
                    j0 = t // 128
                    diag = kb >= 4 * qb
                    if diag:  # diagonal tile: mask its first q-block
                        blk = probT[:, o : o + 128]
                        nc.vector.tensor_mul(blk, blk, tri[:])
                    # masked block's AV last so it doesn't wait on the DVE
                    js = list(range(j0 + 1, 4)) + [j0] if diag else range(4)
                    for j in js:
                        qsub = 4 * qb + j
                        co = o + (j - j0) * 128
                        # The first AV (in emission order) touching each
                        # bank at kb=0 opens its zero region with start=True
                        # (clears has_written for the whole 2KB bank); the
                        # bank's other accumulator then lands on
                        # has_written=0 and overwrites. Only the bank's last
                        # AV carries stop.
                        bank = j // 2
                        start = kb == 0 and bank not in started_banks
                        if kb == 0:
                            started_banks.add(bank)
                        nc.tensor.matmul(
                            po_slice(po, j),
                            probT[:, co : co + 128],
                            v_augf[:, kb * 129 : (kb + 1) * 129],
                            start=start,
                            stop=(j % 2 == 1 and kb == qsub),
                            skip_group_check=True,
                        )
                if g == 2 * qb + 1:  # last group: normalize + store
                    # copy PSUM->SBUF first so the po banks free ASAP (the
                    # next chunk's first AV reuses them), then normalize
                    # from SBUF where DVE runs 2x
                    acc = osbp.tile([128, 2, 258], F32, tag="acc", name="acc")
                    nc.vector.tensor_copy(acc[:, 0, :], po[0][:])
                    nc.vector.tensor_copy(acc[:, 1, :], po[1][:])
                    out_sb = osbp.tile([128, 4, 128], F32, tag="out", name="osb")
                    for j in range(4):
                        aj = acc[:, j // 2, 129 * (j % 2) : 129 * (j % 2) + 129]
                        rcp = smallp.tile([128, 1], F32, tag="rcp", name="rcp")
                        nc.vector.reciprocal(rcp[:], aj[:, 128:129])
                        nc.vector.tensor_scalar_mul(
                            out_sb[:, j, :], aj[:, 0:128], rcp[:]
                        )
                    nc.sync.dma_start(outr[qb, h], out_sb[:])

            # Software-pipelined emission: issue scores(u+1) before the
            # exp-dependent work of unit u so PE never waits on ACT.
            hooks = {
                (0, 3): lambda: emit_wave2(1),
                (1, 3): lambda: emit_wave2(2),
                (2, 3): lambda: emit_wave2(3),
            }
            exp_state = {"ctr": 0}
            pending = []  # 2-deep scores lookahead (ps_s has 3 bufs)
            order = [
                (h, qb) for h in range(GRP) for qb in (3, 2, 1, 0)
            ]  # big chunks first within each head, small-drain tail
            for h, qb in order:
                    if qb == 0:
                        # drain the lookahead before each small chunk: its
                        # diag-heavy groups contend for ps slots with the
                        # queued units (flush fully before the last chunk)
                        keep = 1 if h == GRP - 1 else 2
                        while len(pending) > keep:
                            emit_rest(*pending.pop(0))
                    if (h, qb) in hooks:
                        hooks[(h, qb)]()
                    # packed out accumulators: bank A holds q-subblocks 0,1
                    # at cols [0,129)/[129,258); bank B holds 2,3.
                    po01 = pso.tile([128, 258], F32, tag="o01", name="po01")
                    po23 = pso.tile([128, 258], F32, tag="o23", name="po23")
                    po = (po01, po23)
                    for g in range(2 * qb + 2):
                        scored = emit_scores(h, qb, g)
                        pending.append((h, qb, g, po, scored))
                        if len(pending) > 4:
                            emit_rest(*pending.pop(0))
            for p in pending:
                emit_rest(*p)

    nc.compile()
    return nc


def _get_nc():
    global _CACHED_NC
    if _CACHED_NC is None:
        _CACHED_NC = _build_graph()
    return _CACHED_NC


def _effective_kv(kv, cache, slot):
    """Mirror reference _store_kvcache + gather: returns cache-after-scatter
    gathered at slot positions, shape [B, S, HKV, D]."""
    valid = slot >= 0
    safe = np.where(valid, slot, 0)
    cache = np.array(cache, dtype=np.float32, copy=True)
    val = np.where(valid[:, None, None], kv, cache[safe])
    cache[safe] = val
    return cache[safe.reshape(B, S)]


def _tile_sd(x):
    """[S, D] -> [128, ST, D] with row s at [s % 128, s // 128]."""
    S_, D_ = x.shape
    return np.ascontiguousarray(
        x.reshape(S_ // 128, 128, D_).transpose(1, 0, 2)
    )


def _prep_core_inputs(qb, kk, vv, tri, c):
    bf16 = ml_dtypes.bfloat16
    b, g = c // HKV, c % HKV
    q_sh = qb[b, :, g * GRP : (g + 1) * GRP, :].astype(bf16)  # [S, GRP, D]
    q_tiled = np.stack([_tile_sd(q_sh[:, h, :]) for h in range(GRP)])
    k_tiled = _tile_sd(kk[b, :, g, :].astype(bf16))
    v_sd = vv[b, :, g, :].astype(bf16)  # [S, D]
    v_pad = np.concatenate(
        [v_sd, np.ones((S, 1), dtype=bf16)], axis=1
    )  # ones col baked in
    v_tiled = _tile_sd(v_pad)
    return {"q": q_tiled, "k": k_tiled, "v": v_tiled, "tri": tri}


def kernel(q, k, v, k_cache, v_cache, slot_mapping, batch, seqlen, **_ignored):
    q = np.asarray(q, dtype=np.float32)
    k = np.asarray(k, dtype=np.float32)
    v = np.asarray(v, dtype=np.float32)
    slot = np.asarray(slot_mapping).astype(np.int64)
    assert int(batch) == B and int(seqlen) == S
    assert q.shape == (B * S, H, D)

    kk = _effective_kv(k, k_cache, slot)  # [B, S, HKV, D]
    vv = _effective_kv(v, v_cache, slot)
    qb = q.reshape(B, S, H, D)

    tri = np.triu(np.ones((128, 128), dtype=np.float32)).astype(
        ml_dtypes.bfloat16
    )

    in_maps = [
        _prep_core_inputs(qb, kk, vv, tri, c) for c in range(NCORES)
    ]

    nc = _get_nc()
    res = run_bass_kernel_spmd(nc, in_maps, core_ids=list(range(NCORES)))

    out = np.empty((B, S, H, D), dtype=np.float32)
    for c in range(NCORES):
        b, g = c // HKV, c % HKV
        out[b, :, g * GRP : (g + 1) * GRP, :] = res.results[c]["out"]
    return out.reshape(B * S, H, D)

